# revision 50
# baseline (speedup 1.0000x reference)
"""CoordAtt3D (N,C,D,H,W = 4,64,64,64,64) on 8 Trainium2 NeuronCores.

Sharding: core i owns (sample n = i//2, channel half cs = (i%2)*32), a
[32, 64, 64, 64] slab.  All pools are per-channel (local); the only
cross-core step is the conv1 channel contraction, a 6 KB pair exchange.

v4 — fp16 device I/O (host pre-permutes/converts x by 1024 into fp16's
normal range; s1 absorbs the scale), single HBM pass, ~120 us/rep vs
the 330 us fp32 baseline:
  Host:    x slab -> xin[p=(dl,c), (k,f)] fp16 [128, 65536] where
           d = 4k+dl, f = h*64+w.  Output is the same layout, fp16;
           un-permuted, fp32-cast and /1024 on the host.
  Phase P: 16 chunk DMAs (sync ring only - kept free of sem-blocked
           DMAs so the next rep can prefetch) land straight in a
           21-slot rotating fp16 stash; the 5 spare slots let the next
           rep's input stream run during this rep's exchange/mid.
           Per chunk: PE accumulates quarter-wise d-sums S128[(q,c),
           (hq,w)] in PSUM [128,1024] (2 banks); row-total sums (xd)
           on Act (activation accum_out, 9 chunks) and DVE (fp16
           half-fold + reduce, 7 chunks).
  Mid:     DVE folds S128 -> xh128 [128,16], xw128 [128,64] (FD 1024,
           4x cheaper than a [32,4096] fold); conv1 as 9 small PE
           matmuls on partition-replicated masked weights (row-offset
           matmuls are rejected by walrus); pair AllGather + add (all
           mid DMAs on the scalar ring); BN + hardswish; three tiny
           convs on 128 partitions via replicated weights; sigmoid on
           Act; M4[p,(h,w)] = ah*aw built by one DVE tensor_tensor.
  Phase M: obuf = stash[k] * ad[p,k] * M4 (DVE fp16 2x), output DMAs
           split across the Act-HWDGE and GpSimd-SWDGE rings (the
           write side is ring-limited).
"""

import os
import sys

if "/opt/trn_rl_repo" not in sys.path:
    sys.path.insert(0, "/opt/trn_rl_repo")

import numpy as np

_DEBUG_STAGE = os.environ.get("KERNEL_DEBUG_STAGE", "full")

N, C, D, H, W = 4, 64, 64, 64, 64
MIP = 8
BN_EPS = 1e-5
NCORES = 8
CL = C // 2          # 32 channels per core
DL = 4               # d-values per chunk
NK = D // DL         # 16 chunks
FREE = H * W         # 4096
P = 128
NQ = 4               # h quarters
HQ = H // NQ         # 16
SFREE = HQ * W       # 1024 free elems of S128 per partition

# row-total sums (xd) split 9/7 Act/DVE; heavier Act shares lag the
# (fast) input stream and delay conv1, lighter ones overload DVE
DVE_KS = frozenset((1, 3, 5, 7, 9, 11, 13))  # row-sum chunks on DVE
GP_MKS = frozenset()                  # phase-M on GpSimd unsupported
                                      # (walrus rejects Pool TensorScalarPtr)
XSCALE = 1024.0      # host-side x scale to keep fp16 I/O in normal range
NSLOT = NK + 5       # stash ring: 5 spare slots let the next rep's input
                     # prefetch run during this rep's exchange/mid phase

_PROGS = {}


def _build_program(reps=1, stage=None):
    if stage is None:
        stage = _DEBUG_STAGE
    if (stage, reps) in _PROGS:
        return _PROGS[(stage, reps)]

    import concourse.bacc as bacc
    import concourse.mybir as mybir
    import concourse.tile as tile

    fp32 = mybir.dt.float32
    fp16 = mybir.dt.float16
    bf16 = mybir.dt.bfloat16
    AX = mybir.AxisListType
    OP = mybir.AluOpType
    AF = mybir.ActivationFunctionType

    nc = bacc.Bacc("TRN2", target_bir_lowering=False, debug=False,
                   num_devices=NCORES)

    xint = nc.dram_tensor("xin", [P, NK * FREE], fp16, kind="ExternalInput")
    w1t = nc.dram_tensor("w1rep", [P, MIP], fp32, kind="ExternalInput")
    w1mt = nc.dram_tensor("w1m", [P, DL * MIP], fp32, kind="ExternalInput")
    wdt = nc.dram_tensor("wd128", [MIP, P], fp32, kind="ExternalInput")
    wht = nc.dram_tensor("wh128", [MIP, P], fp32, kind="ExternalInput")
    wwt = nc.dram_tensor("ww128", [MIP, P], fp32, kind="ExternalInput")
    bdt = nc.dram_tensor("bd", [P, 1], fp32, kind="ExternalInput")
    bht = nc.dram_tensor("bh", [P, 1], fp32, kind="ExternalInput")
    bwt = nc.dram_tensor("bw", [P, 1], fp32, kind="ExternalInput")
    s1t = nc.dram_tensor("s1", [MIP, 1], fp32, kind="ExternalInput")
    t1t = nc.dram_tensor("t1", [MIP, 1], fp32, kind="ExternalInput")
    onest = nc.dram_tensor("ones16", [P, CL], fp16, kind="ExternalInput")
    # x (and hence out) carry a host-side 1024x scale so fp16 stays in its
    # normal range down to the 1e-6 output magnitudes that matter; the BN
    # scale s1 absorbs the factor and the host divides the output by 1024
    outt = nc.dram_tensor("out", [P, NK * FREE], fp16,
                          kind="ExternalOutput")
    if stage == "copyin":
        outt = None  # unused
    dbgt = nc.dram_tensor("dbg", [P, 256], fp32, kind="ExternalOutput")

    with tile.TileContext(nc) as tc:
        with tc.tile_pool(name="const", bufs=1) as cp, \
             tc.tile_pool(name="acc", bufs=1) as accp, \
             tc.tile_pool(name="small", bufs=1) as smp, \
             tc.tile_pool(name="psum", bufs=1, space="PSUM") as pp, \
             tc.tile_pool(name="dram", bufs=1, space="DRAM") as dp:

            if stage in ("copy", "copyin", "copyout", "copy2"):
                stash = [accp.tile([P, FREE], fp16, name=f"stash{k}")
                         for k in range(NK)]
                for _ in range(reps):
                    for k in range(NK):
                        if stage != "copyout":
                            nc.sync.dma_start(
                                stash[k][:], xint[:, k * FREE:(k + 1) * FREE])
                        if stage == "copy":
                            nc.scalar.dma_start(
                                outt[:, k * FREE:(k + 1) * FREE], stash[k][:])
                        elif stage == "copyout":
                            eng = nc.scalar if k % 2 == 0 else nc.sync
                            eng.dma_start(
                                outt[:, k * FREE:(k + 1) * FREE], stash[k][:])
                        elif stage == "copy2":
                            eng = nc.scalar if k % 2 == 0 else nc.gpsimd
                            eng.dma_start(
                                outt[:, k * FREE:(k + 1) * FREE], stash[k][:])
            else:
                # consts: loaded once per NEFF execution
                w1s = cp.tile([P, MIP], fp32)
                nc.sync.dma_start(w1s[:], w1t[:])
                w1ms = cp.tile([P, DL * MIP], fp32)
                nc.sync.dma_start(w1ms[:], w1mt[:])
                wds = cp.tile([MIP, P], fp32)
                nc.sync.dma_start(wds[:], wdt[:])
                whs = cp.tile([MIP, P], fp32)
                nc.sync.dma_start(whs[:], wht[:])
                wws = cp.tile([MIP, P], fp32)
                nc.sync.dma_start(wws[:], wwt[:])
                bds = cp.tile([P, 1], fp32)
                nc.sync.dma_start(bds[:], bdt[:])
                bhs = cp.tile([P, 1], fp32)
                nc.sync.dma_start(bhs[:], bht[:])
                bws = cp.tile([P, 1], fp32)
                nc.sync.dma_start(bws[:], bwt[:])
                s1s = cp.tile([MIP, 1], fp32)
                nc.sync.dma_start(s1s[:], s1t[:])
                t1s = cp.tile([MIP, 1], fp32)
                nc.sync.dma_start(t1s[:], t1t[:])
                ones = cp.tile([P, CL], fp16)
                nc.sync.dma_start(ones[:], onest[:])
                half_b = cp.tile([MIP, 1], fp32)
                nc.vector.memset(half_b[:], 0.5)
                # trigger the sigmoid table-set load while Act is idle
                dums = cp.tile([MIP, 1], fp32)
                nc.scalar.activation(dums[:], s1s[:], AF.Sigmoid)

                if stage == "att0":
                    # keep has_collectives (and the NEFF entry barrier)
                    # comparable with the collective stages
                    din = dp.tile([1, 4], fp32)
                    dout = dp.tile([2, 4], fp32)
                    nc.gpsimd.collective_compute(
                        "AllGather", OP.bypass,
                        replica_groups=[[0, 1], [2, 3], [4, 5], [6, 7]],
                        ins=[din[:].opt()], outs=[dout[:].opt()])

                consts = (w1s, w1ms, wds, whs, wws, bds, bhs, bws, s1s, t1s,
                          ones, half_b)
                slots = [accp.tile([P, FREE], fp16, name=f"stash{s}")
                         for s in range(NSLOT)]
                for rep in range(reps):
                    _body(nc, tc, stage, rep, slots, accp, smp, pp, dp,
                          fp32, fp16, bf16, AX, OP, AF, consts,
                          xint, outt, dbgt)

    nc.compile()
    _PROGS[(stage, reps)] = nc
    return nc


def _body(nc, tc, stage, rep, slots, accp, smp, pp, dp, fp32, fp16, bf16,
          AX, OP, AF, consts, xint, outt, dbgt):
    (w1s, w1ms, wds, whs, wws, bds, bhs, bws, s1s, t1s, ones,
     half_b) = consts

    stash = [slots[(rep * NK + k) % NSLOT] for k in range(NK)]
    xd_all = accp.tile([P, NK], fp32)
    xh128 = accp.tile([P, HQ], fp32)
    xw128 = accp.tile([P, W], fp32)
    vscr = accp.tile([P, FREE // 2], fp16)  # DVE fold scratch
    S = pp.tile([P, SFREE], fp32, tag="S")

    # ---------------- Phase P: stream chunks, pooled sums ----------------
    # PE:  S[(q,c), (hq,w)] += sum_dl stash[k][(dl,c), (q,hq,w)]
    # Act: xd_all[p, k] = accum_out of in-place identity pass (12 chunks)
    # DVE: fp16 fold-fold-reduce (4 chunks)
    for k in range(NK):
        nc.sync.dma_start(stash[k][:], xint[:, k * FREE:(k + 1) * FREE])
        for q in range(NQ):
            for b in range(2):
                lo = q * SFREE + b * 512
                nc.tensor.matmul(
                    S[q * CL:(q + 1) * CL, b * 512:(b + 1) * 512],
                    ones[:], stash[k][:, lo:lo + 512],
                    start=(k == 0), stop=(k == NK - 1),
                    tile_position=(0, q * CL))
        if k not in DVE_KS:
            # in-place identity: values unchanged, frees the 8 KB scratch
            # (one more prefetch slot); Act ordering after PE reads is
            # absorbed by Act's slack
            nc.scalar.activation(stash[k][:], stash[k][:], AF.Identity,
                                 accum_out=xd_all[:, k:k + 1])
        else:
            nc.vector.tensor_tensor(vscr[:], stash[k][:, :FREE // 2],
                                    stash[k][:, FREE // 2:], op=OP.add)
            nc.vector.tensor_reduce(xd_all[:, k:k + 1], vscr[:],
                                    axis=AX.X, op=OP.add)

    # ---------------- fold S -> xh128 [p, hq], xw128 [p, w] ----------------
    nc.vector.tensor_reduce(
        xh128[:], S[:].rearrange("p (hq w) -> p hq w", w=W),
        axis=AX.X, op=OP.add)
    nc.vector.tensor_reduce(
        xw128[:], S[:].rearrange("p (hq w) -> p w hq", w=W),
        axis=AX.X, op=OP.add)

    if stage == "pool":
        nc.sync.dma_start(dbgt[:, 0:NK], xd_all[:])
        nc.sync.dma_start(dbgt[:, 32:32 + HQ], xh128[:])
        nc.sync.dma_start(dbgt[:, 64:64 + W], xw128[:])
        return

    # ---------------- conv1: py1[m, 192] in PSUM ----------------
    # cols 0:64   d-section, col = dl*16 + k  (d = 4k+dl, permuted)
    # cols 64:128 h-section, col = h          (natural)
    # cols 128:192 w-section
    # w1ms[:, g*8:(g+1)*8] is w1rep zero-masked to partition group g, so
    # every matmul contracts the full 128 partitions (no row offsets)
    py1 = pp.tile([MIP, 192], fp32, tag="py1")
    for dl in range(DL):
        nc.tensor.matmul(py1[:, dl * NK:(dl + 1) * NK],
                         w1ms[:, dl * MIP:(dl + 1) * MIP], xd_all[:])
    for q in range(NQ):
        nc.tensor.matmul(py1[:, 64 + q * HQ:64 + (q + 1) * HQ],
                         w1ms[:, q * MIP:(q + 1) * MIP], xh128[:])
    nc.tensor.matmul(py1[:, 128:192], w1s[:], xw128[:])
    y1 = smp.tile([MIP, 192], fp32, tag="y1")
    nc.vector.tensor_copy(y1[:], py1[:])

    if stage == "conv1":
        nc.sync.dma_start(dbgt[0:MIP, 0:192], y1[:])
        return

    # ---------------- pair exchange ----------------
    # NOTE: all mid-phase DMAs ride the scalar (Act) ring so the sync ring
    # stays free for the next rep's stash prefetch (HWDGE rings are FIFO
    # per issuing engine — a sem-blocked DMA would head-of-line-block it).
    y1r = smp.tile([MIP, 192], fp32, tag="y1r")
    if stage == "att0":
        nc.vector.tensor_copy(y1r[:], y1[:])  # timing-only: skip exchange
    elif stage == "nocc":
        cin = dp.tile([MIP, 192], fp32)
        cout = dp.tile([MIP, 192], fp32)
        nc.scalar.dma_start(cin[:], y1[:])
        nc.scalar.dma_start(cout[:], cin[:])
        nc.scalar.dma_start(y1r[:], cout[:])
    elif stage == "ccar":
        # pair AllReduce (CCE in-stream add): one return DMA, no DVE add
        cin = dp.tile([MIP, 192], fp32)
        cout = dp.tile([MIP, 192], fp32)
        nc.scalar.dma_start(cin[:], y1[:])
        nc.gpsimd.collective_compute(
            "AllReduce", OP.add,
            replica_groups=[[0, 1], [2, 3], [4, 5], [6, 7]],
            ins=[cin[:].opt()], outs=[cout[:].opt()])
        nc.scalar.dma_start(y1r[:], cout[:])
    else:
        # pair AllGather + local add (order-symmetric)
        cin = dp.tile([MIP, 192], fp32)
        cout = dp.tile([2, MIP, 192], fp32)
        nc.scalar.dma_start(cin[:], y1[:])
        nc.gpsimd.collective_compute(
            "AllGather", OP.bypass,
            replica_groups=[[0, 1], [2, 3], [4, 5], [6, 7]],
            ins=[cin[:].opt()], outs=[cout[:].opt()])
        if stage == "ccag2":
            cout2 = dp.tile([2, MIP, 192], fp32)
            nc.gpsimd.collective_compute(
                "AllGather", OP.bypass,
                replica_groups=[[0, 1], [2, 3], [4, 5], [6, 7]],
                ins=[cin[:].opt()], outs=[cout2[:].opt()])
        yg = smp.tile([MIP, 2 * 192], fp32, tag="yg")
        nc.scalar.dma_start(yg[:].rearrange("m (a f) -> m a f", a=2),
                            cout[:].rearrange("a m f -> m a f"))
        nc.vector.tensor_tensor(y1r[:], yg[:, 0:192], yg[:, 192:384],
                                op=OP.add)

    # ---------------- BN (folded, incl /4096) + hardswish ----------------
    ybn = smp.tile([MIP, 192], fp32, tag="ybn")
    nc.scalar.activation(ybn[:], y1r[:], AF.Identity,
                         bias=t1s[:], scale=s1s[:])
    hs = smp.tile([MIP, 192], fp32, tag="hs")
    # relu(v/6 + 0.5) == relu6(v+3)/6 before the min-with-1 clamp
    nc.scalar.activation(hs[:], ybn[:], AF.Relu,
                         bias=half_b[:], scale=1.0 / 6.0)
    nc.vector.tensor_scalar_min(hs[:], hs[:], 1.0)
    yact = smp.tile([MIP, 192], fp32, tag="yact")
    nc.vector.tensor_tensor(yact[:], ybn[:], hs[:], op=OP.mult)

    # ---------------- three tiny convs + sigmoid, on 128 partitions ------
    ad_pm = accp.tile([P, NK], fp32)   # ad_pm[(dl,c), k] = ad[c, 4k+dl]
    pad = pp.tile([P, 64], fp32, tag="pad")
    nc.tensor.matmul(pad[:], wds[:], yact[:, 0:64])
    for dl in range(DL):
        sl = slice(dl * CL, (dl + 1) * CL)
        nc.scalar.activation(ad_pm[sl, :], pad[sl, dl * NK:(dl + 1) * NK],
                             AF.Sigmoid, bias=bds[sl, :], scale=1.0)
    ah = accp.tile([P, H], fp16)
    pah = pp.tile([P, 64], fp32, tag="pah")
    nc.tensor.matmul(pah[:], whs[:], yact[:, 64:128])
    nc.scalar.activation(ah[:], pah[:], AF.Sigmoid, bias=bhs[:], scale=1.0)
    aw = accp.tile([P, W], fp16)
    paw = pp.tile([P, 64], fp32, tag="paw")
    nc.tensor.matmul(paw[:], wws[:], yact[:, 128:192])
    nc.scalar.activation(aw[:], paw[:], AF.Sigmoid, bias=bws[:], scale=1.0)

    if stage == "mid":
        nc.sync.dma_start(dbgt[:, 0:NK], ad_pm[:])
        adbg = accp.tile([P, 128], fp32)
        nc.vector.tensor_copy(adbg[:, 0:64], ah[:])
        nc.vector.tensor_copy(adbg[:, 64:128], aw[:])
        nc.sync.dma_start(dbgt[:, 32:160], adbg[:])
        return

    # ---------------- M4[p, (h,w)] = ah[p,h] * aw[p,w] ----------------
    M4 = accp.tile([P, FREE], fp16)
    m4eng = nc.gpsimd if stage == "m4gp" else nc.vector
    m4eng.tensor_tensor(
        M4[:].rearrange("p (h w) -> p h w", w=W),
        ah[:].unsqueeze(2).broadcast_to([P, H, W]),
        aw[:].unsqueeze(1).broadcast_to([P, H, W]),
        op=OP.mult)

    # ---------------- Phase M: fused broadcast multiply ----------------
    # output DMAs split across the Act-HWDGE and GpSimd-SWDGE rings (the
    # write side is ring-limited: 2-ring split measures ~6 us faster)
    for k in range(NK):
        obuf = smp.tile([P, FREE], fp16, tag="io", bufs=2)
        ceng = nc.gpsimd if k in GP_MKS else nc.vector
        ceng.scalar_tensor_tensor(
            obuf[:], stash[k][:], ad_pm[:, k:k + 1], M4[:],
            op0=OP.mult, op1=OP.mult)
        eng = nc.scalar if k % 2 == 0 else nc.gpsimd
        eng.dma_start(outt[:, k * FREE:(k + 1) * FREE], obuf[:])


def _host_inputs(x, conv1_w, conv1_b, bn_gamma, bn_beta, bn_mean, bn_var,
                 convd_w, convd_b, convh_w, convh_b, convw_w, convw_b):
    scale = bn_gamma / np.sqrt(bn_var + BN_EPS)
    s1 = (scale / float(FREE * XSCALE)).astype(np.float32).reshape(MIP, 1)
    t1 = ((conv1_b - bn_mean) * scale + bn_beta).astype(np.float32) \
        .reshape(MIP, 1)
    ones16 = (np.arange(P)[:, None] % CL ==
              np.arange(CL)[None, :]).astype(np.float16)
    in_maps = []
    for i in range(NCORES):
        n, half = i // 2, i % 2
        cs = half * CL
        # [c, (k dl), h, w] -> [p=(dl,c), (k,f)], scaled by XSCALE
        xs = x[n, cs:cs + CL].reshape(CL, NK, DL, FREE)
        xin = np.ascontiguousarray(
            (xs.transpose(2, 0, 1, 3) * np.float32(XSCALE))
            .astype(np.float16)
        ).reshape(P, NK * FREE)
        w1rep = np.tile(conv1_w[:, cs:cs + CL].T, (DL, 1)).astype(np.float32)
        w1m = np.zeros((P, DL * MIP), np.float32)
        for g in range(DL):
            w1m[g * CL:(g + 1) * CL, g * MIP:(g + 1) * MIP] = \
                w1rep[g * CL:(g + 1) * CL]
        in_maps.append({
            "xin": xin,
            "w1rep": np.ascontiguousarray(w1rep),
            "w1m": w1m,
            "wd128": np.ascontiguousarray(
                np.tile(convd_w[cs:cs + CL, :].T, (1, DL))),
            "wh128": np.ascontiguousarray(
                np.tile(convh_w[cs:cs + CL, :].T, (1, DL))),
            "ww128": np.ascontiguousarray(
                np.tile(convw_w[cs:cs + CL, :].T, (1, DL))),
            "bd": np.ascontiguousarray(
                np.tile(convd_b[cs:cs + CL], DL).reshape(P, 1)),
            "bh": np.ascontiguousarray(
                np.tile(convh_b[cs:cs + CL], DL).reshape(P, 1)),
            "bw": np.ascontiguousarray(
                np.tile(convw_b[cs:cs + CL], DL).reshape(P, 1)),
            "s1": s1,
            "t1": t1,
            "ones16": ones16,
        })
    return in_maps


def _run(in_maps, trace=False):
    from concourse.bass_utils import run_bass_kernel_spmd
    nc = _build_program()
    return run_bass_kernel_spmd(nc, in_maps, list(range(NCORES)),
                                trace=trace)


def kernel(**inputs):
    args = {k: np.asarray(v, dtype=np.float32) for k, v in inputs.items()}
    in_maps = _host_inputs(**args)
    res = _run(in_maps)
    y = np.empty((N, C, D, H, W), dtype=np.float32)
    for i in range(NCORES):
        n, half = i // 2, i % 2
        cs = half * CL
        o = res.results[i]["out"].reshape(DL, CL, NK, FREE)
        y[n, cs:cs + CL] = o.transpose(1, 2, 0, 3) \
            .astype(np.float32).reshape(CL, D, H, W)
    y *= np.float32(1.0 / XSCALE)
    return y
